# revision 19
# baseline (speedup 1.0000x reference)
"""Trainium2 Bass kernel for windowed attention with relative-position bias.

Problem (hardcoded): x [32, 256, 25, 25] f32, w_qkv [256, 768], rel_emb [2401, 8],
w_out [256, 256], rel_idx [625, 625] int32. 8 heads of dim 32, n = 625 tokens.

Sharding: data-parallel over batch; 4 batches per core on 8 NeuronCores; weights
and bias replicated. No collectives.

Per-core dataflow (all matmuls bf16, f32 PSUM accumulate):
  qkv^T = w_qkv^T @ x          -> q^T,k^T tiles [32h..., 625]  (q pre-scaled on host)
  v     = x^T @ w_v            -> v tiles [125, jt, h, 64]: dh cols, a ones column
                                  (softmax denominator), zero padding to 64 so the
                                  av psum tile is fully initialized
  sim^T = k_h^T q_h            per (head-pair, j-tile) round; the two heads use
                               adjacent PE row-groups for tile concurrency
  expsim = exp(sim^T)          (ScalarE, PSUM -> SBUF bf16)
  es     = expsim * exp(bias)  (DVE/gpsimd, 640-padded tiles for the 2x bf16 mode)
  av^T   = [v|1|0]^T @ es      (col-tiled pair; accumulated over j tiles in PSUM)
  1/den                        one strided-partition DVE reciprocal_approx_fast per
                               pair straight off the av psum rows; bf16 recip rows
                               broadcast across partitions via a DRAM-hop DMA
  outT   = raw av^T copies (DVE, frees the av bank fast), then deferred bf16
           multiplies by the broadcast recip rows; project with w_out -> HBM
Batches are software-pipelined: batch b's x-load and qkv matmuls are issued at
staggered points inside batch b-1's attention rounds.
"""

import sys

if "/opt/trn_rl_repo" not in sys.path:
    sys.path.insert(0, "/opt/trn_rl_repo")

import numpy as np
import ml_dtypes

B, D, WS = 32, 256, 25
N = WS * WS            # 625
NP = 640               # padded row length (4B-aligned bf16 rows)
H, DH = 8, 32
NC = 8                 # cores
BL = B // NC           # 4 batches per core
SCALE = DH ** -0.5
JT = 5                 # j tiles of 125
JP = N // JT           # 125
CHUNKS = ((0, 512), (512, 113))   # i chunks, bank-aligned (matmul must not cross a 512-f32 PSUM bank)

_cache = {}


def _build():
    import concourse.bass as bass
    from concourse import bacc, mybir
    from concourse.tile import TileContext

    f32 = mybir.dt.float32
    bf16 = mybir.dt.bfloat16

    nc = bacc.Bacc()
    x_ext = nc.declare_dram_parameter("x", [BL, D, N], bf16, isOutput=False)
    wqkv_ext = nc.declare_dram_parameter("wqkv", [D, 3 * D], bf16, isOutput=False)
    wout_ext = nc.declare_dram_parameter("wout", [D, D], bf16, isOutput=False)
    biasT_ext = nc.declare_dram_parameter("biasT", [H, JT, JP, NP], bf16, isOutput=False)
    out_ext = nc.declare_dram_parameter("out", [BL, D, N], f32, isOutput=True)

    with TileContext(nc) as tc:
        with (
            tc.tile_pool(name="const", bufs=1) as const,
            tc.tile_pool(name="xp", bufs=2) as xp,
            tc.tile_pool(name="qk", bufs=2) as qkp,
            tc.tile_pool(name="vp", bufs=2) as vp,
            tc.tile_pool(name="es", bufs=4) as esp,
            tc.tile_pool(name="dn", bufs=4) as dnp,
            tc.tile_pool(name="avs", bufs=2) as avsp,
            tc.tile_pool(name="rb", bufs=4) as rbp,
            tc.tile_pool(name="ot", bufs=2) as otp,
            tc.tile_pool(name="res", bufs=2) as resp,
            tc.tile_pool(name="sim", bufs=2, space="PSUM") as simp,
            tc.tile_pool(name="avb", bufs=1, space="PSUM") as avp,
            tc.tile_pool(name="acc", bufs=1, space="PSUM") as accp,
        ):
            wqkv_sb = const.tile([128, 2, 3 * D], bf16)
            nc.sync.dma_start(out=wqkv_sb, in_=wqkv_ext.rearrange("(k p) c -> p k c", p=128))
            wout_sb = const.tile([128, 2, D], bf16)
            nc.sync.dma_start(out=wout_sb, in_=wout_ext.rearrange("(k p) c -> p k c", p=128))
            biasT_sb = const.tile([JP, H, JT, NP], bf16)
            for h in range(H):
                for jt in range(JT):
                    nc.sync.dma_start(out=biasT_sb[:, h, jt, :], in_=biasT_ext[h, jt])

            def x_load(b):
                x_sb = xp.tile([128, 2, N], bf16, tag="x", name=f"x_sb_{b}")
                nc.sync.dma_start(out=x_sb, in_=x_ext[b].rearrange("(k p) n -> p k n", p=128))
                return x_sb

            def qkv_pieces(b, x_sb):
                """Closures, each issuing one slice of batch b's qkv work."""
                qkT_sb = qkp.tile([128, 4, NP], bf16, tag="qkT", name=f"qkT_{b}")
                v_sb = vp.tile([JP, JT, H, 2 * DH], bf16, tag="v", name=f"v_sb_{b}")

                def do_m(m):
                    ps = accp.tile([128, NP], f32, tag="acc", name=f"ps_{b}_{m}")
                    for kt in range(2):
                        for lo, sz in CHUNKS:
                            nc.tensor.matmul(
                                ps[:, lo:lo + sz],
                                wqkv_sb[:, kt, m * 128:(m + 1) * 128],
                                x_sb[:, kt, lo:lo + sz],
                                start=(kt == 0), stop=(kt == 1))
                    nc.vector.tensor_copy(qkT_sb[:, m, :N], ps[:, :N])

                def do_v(nt):
                    psv = accp.tile([128, NP], f32, tag="acc", name=f"psv_{b}_{nt}")
                    for kt in range(2):
                        nc.tensor.matmul(
                            psv[0:JP, :D],
                            x_sb[:, kt, nt * JP:(nt + 1) * JP],
                            wqkv_sb[:, kt, 2 * D:3 * D],
                            start=(kt == 0), stop=(kt == 1))
                    nc.vector.tensor_copy(
                        v_sb[:, nt, :, 0:DH],
                        psv[0:JP, :D].rearrange("p (h d) -> p h d", h=H))

                def do_consts():
                    nc.gpsimd.memset(v_sb[:, :, :, DH:DH + 1], 1.0)
                    nc.gpsimd.memset(v_sb[:, :, :, DH + 1:2 * DH], 0.0)

                pieces = [lambda m=m: do_m(m) for m in range(4)]
                pieces += [lambda nt=nt: do_v(nt) for nt in range(JT)]
                pieces.append(do_consts)
                return (b, qkT_sb, v_sb), pieces

            def attention_phase(ctx, hooks=None):
                """Attention + projection for a prepared batch. hooks[r] (if set)
                is invoked after round r to interleave the next batch's work."""
                b, qkT_sb, v_sb = ctx
                hooks = hooks or {}
                outT_sb = otp.tile([128, 2, NP], bf16, tag="outT", name=f"outT_{b}")

                av_box = [None]
                muls_pend = []

                def finish_pair(p, av):
                    # raw av -> SBUF f32 (frees the av psum bank quickly);
                    # custom DVE ops are SBUF-only, so 1/den reads the copy
                    av_sb = avsp.tile([128, 2, 512], f32, tag="avsb", name=f"avsb_{b}_{p}")
                    nc.vector.tensor_copy(av_sb[:, 0, :], av[:, 0, :])
                    nc.vector.tensor_copy(av_sb[:, 1, 0:113], av[:, 1, 0:113])
                    rbs = []
                    for s in range(2):
                        dr = av_sb[64 * s + DH:64 * s + DH + 1]
                        rcp_f = dnp.tile([1, 2, 512], f32, tag="rcpf",
                                         name=f"rcpf_{b}_{p}_{s}")
                        nc.vector.reciprocal(rcp_f[:, 0, :], dr[:, 0, :])
                        nc.vector.reciprocal(rcp_f[:, 1, 0:113], dr[:, 1, 0:113])
                        rcp_b = dnp.tile([1, 2, 512], bf16, tag="rcpb",
                                         name=f"rcpb_{b}_{p}_{s}")
                        nc.vector.memset(rcp_b[:, 1, 113:512], 1.0)
                        nc.vector.tensor_copy(rcp_b[:, 0, :], rcp_f[:, 0, :])
                        nc.vector.tensor_copy(rcp_b[:, 1, 0:113], rcp_f[:, 1, 0:113])
                        rb_s = rbp.tile([128, 2, 512], bf16, tag="rb",
                                        name=f"rb_{b}_{p}_{s}")
                        nc.gpsimd.partition_broadcast(rb_s, rcp_b, channels=128)
                        rbs.append(rb_s)
                    muls_pend.append((p, av_sb, rbs))

                def issue_muls():
                    p, av_sb, rbs = muls_pend.pop(0)
                    for s in range(2):
                        h = 2 * p + s
                        hq, mt = (h % 4) * 32, h // 4
                        for ci, (lo, sz) in enumerate(CHUNKS):
                            nc.vector.tensor_mul(
                                outT_sb[hq:hq + DH, mt, lo:lo + sz],
                                av_sb[64 * s:64 * s + DH, ci, 0:sz],
                                rbs[s][64 * s:64 * s + DH, ci, 0:sz])

                def issue_av(p, jt, es_pair):
                    if jt == 0:
                        av_box[0] = avp.tile([128, 2, 512], f32, tag="av",
                                             name=f"av_{b}_{p}")
                    av = av_box[0]
                    for ci, (lo, sz) in enumerate(CHUNKS):
                        for s in range(2):
                            nc.tensor.matmul(
                                av[64 * s:64 * s + 2 * DH, ci, 0:sz],
                                v_sb[0:JP, jt, 2 * p + s, :],
                                es_pair[s][0:JP, lo:lo + sz],
                                start=(jt == 0), stop=(jt == JT - 1),
                                tile_position=(0, 64 * s),
                                skip_group_check=True)
                    if jt == JT - 1:
                        finish_pair(p, av)

                rounds = [(p, jt) for p in range(4) for jt in range(JT)]
                pend = []
                for r, (p, jt) in enumerate(rounds):
                    if r % JT == 3 and muls_pend:
                        issue_muls()
                    sims = [simp.tile([JP, NP], f32, tag="sim", name=f"sim_{b}_{r}_{s}")
                            for s in range(2)]
                    for ci, (lo, sz) in enumerate(CHUNKS):
                        for s in range(2):
                            h = 2 * p + s
                            hq, mt = (h % 4) * 32, h // 4
                            nc.tensor.matmul(
                                sims[s][:, lo:lo + sz],
                                qkT_sb[hq:hq + 32, 2 + mt, jt * JP:(jt + 1) * JP],
                                qkT_sb[hq:hq + 32, mt, lo:lo + sz],
                                start=True, stop=True, tile_position=(hq, 0))
                    es_pair = []
                    for s in range(2):
                        h = 2 * p + s
                        esr = esp.tile([JP, NP], bf16, tag="esr", name=f"esr_{b}_{r}_{s}")
                        nc.scalar.activation(out=esr[:, :N], in_=sims[s][:, :N],
                                             func=mybir.ActivationFunctionType.Exp)
                        es = esp.tile([JP, NP], bf16, tag="es", name=f"es_{b}_{r}_{s}")
                        eng = nc.gpsimd if (r + s) % 5 == 4 else nc.vector
                        eng.tensor_mul(es[:, :N], esr[:, :N],
                                       biasT_sb[0:JP, h, jt, :N])
                        es_pair.append(es)
                    pend.append((p, jt, es_pair))
                    if len(pend) > 1:
                        pp, pjt, pes = pend.pop(0)
                        issue_av(pp, pjt, pes)
                    if r in hooks:
                        hooks[r]()
                while pend:
                    pp, pjt, pes = pend.pop(0)
                    issue_av(pp, pjt, pes)
                while muls_pend:
                    issue_muls()

                # output projection: out^T[c, i] = sum_d wout[d, c] outT[d, i]
                for ct in range(2):
                    psp = accp.tile([128, NP], f32, tag="acc", name=f"psp_{b}_{ct}")
                    for kt in range(2):
                        for lo, sz in CHUNKS:
                            nc.tensor.matmul(
                                psp[:, lo:lo + sz],
                                wout_sb[:, kt, ct * 128:(ct + 1) * 128],
                                outT_sb[:, kt, lo:lo + sz],
                                start=(kt == 0), stop=(kt == 1))
                    o_t = resp.tile([128, NP], f32, tag="ot", name=f"o_t_{b}_{ct}")
                    nc.scalar.copy(o_t[:, :N], psp[:, :N])
                    nc.sync.dma_start(out=out_ext[b, ct * 128:(ct + 1) * 128, :],
                                      in_=o_t[:, :N])

            # software pipeline across batches: x-load one round in, then the
            # ten qkv pieces spread over the middle rounds of the prior batch
            x0 = x_load(0)
            ctx, pieces = qkv_pieces(0, x0)
            for piece in pieces:
                piece()
            for b in range(1, BL + 1):
                if b < BL:
                    hooks = {}
                    box = {}

                    def mk_xload(bb=b, box=box):
                        box["x"] = x_load(bb)

                    def mk_qkv(bb=b, box=box):
                        box["ctx"], box["pieces"] = qkv_pieces(bb, box["x"])
                        box["pieces"][0]()

                    hooks[1] = mk_xload
                    hooks[4] = mk_qkv
                    for i in range(1, 10):
                        def run_piece(i=i, box=box):
                            box["pieces"][i]()
                        hooks[4 + i] = run_piece
                    attention_phase(ctx, hooks)
                    ctx = box["ctx"]
                else:
                    attention_phase(ctx)

    nc.compile()
    return nc


def _get_nc():
    if "nc" not in _cache:
        _cache["nc"] = _build()
    return _cache["nc"]


def make_in_maps(x, w_qkv, rel_emb, w_out, rel_idx):
    bf = ml_dtypes.bfloat16
    wqkv_s = np.array(w_qkv, dtype=np.float32, copy=True)
    wqkv_s[:, :D] *= SCALE                      # fold q scaling into weights
    wqkv_b = wqkv_s.astype(bf)
    wout_b = np.asarray(w_out, dtype=np.float32).astype(bf)
    # bias[h, i, j] = rel_emb[rel_idx[i, j], h];  biasT[h, j, i] = bias[h, i, j]
    bias = np.asarray(rel_emb, dtype=np.float32)[np.asarray(rel_idx)]   # [i, j, h]
    ebiasT = np.exp(np.ascontiguousarray(bias.transpose(2, 1, 0)))      # [h, j, i]
    biasT = np.zeros((H, JT, JP, NP), dtype=np.float32)
    biasT[..., :N] = ebiasT.reshape(H, JT, JP, N)
    biasT = biasT.astype(bf)
    xf = np.asarray(x, dtype=np.float32).reshape(B, D, N).astype(bf)
    return [
        {"x": xf[c * BL:(c + 1) * BL], "wqkv": wqkv_b, "wout": wout_b, "biasT": biasT}
        for c in range(NC)
    ]


def kernel(x, w_qkv, rel_emb, w_out, rel_idx):
    from concourse.bass_utils import run_bass_kernel_spmd

    nc = _get_nc()
    in_maps = make_in_maps(x, w_qkv, rel_emb, w_out, rel_idx)
    res = run_bass_kernel_spmd(nc, in_maps, list(range(NC)))
    out = np.concatenate([res.results[c]["out"] for c in range(NC)], axis=0)
    return out.reshape(B, D, WS, WS).astype(np.float32)


# revision 24
# speedup vs baseline: 1.3423x; 1.3423x over previous
"""Trainium2 Bass kernel for windowed attention with relative-position bias.

Problem (hardcoded): x [32, 256, 25, 25] f32, w_qkv [256, 768], rel_emb [2401, 8],
w_out [256, 256], rel_idx [625, 625] int32. 8 heads of dim 32, n = 625 tokens.

Sharding: data-parallel over batch; 4 batches per core on 8 NeuronCores; weights
and bias replicated. No collectives.

Per-core dataflow (all matmuls bf16, f32 PSUM accumulate):
  qkv^T = w_qkv^T @ x          -> q^T,k^T tiles [32h..., 625]  (q pre-scaled on host)
  v     = x^T @ w_v            -> v tiles [125, jt, h, 64]: dh cols, a ones column
                                  (softmax denominator), zero padding to 64 so the
                                  av psum tile is fully initialized
  sim^T = k_h^T q_h            per (head-pair, j-tile) round; the two heads use
                               adjacent PE row-groups for tile concurrency
  expsim = exp(sim^T)          (ScalarE, PSUM -> SBUF bf16)
  es     = expsim * exp(bias)  (DVE/gpsimd, 640-padded tiles for the 2x bf16 mode)
  av^T   = [v|1|0]^T @ es      (col-tiled pair; accumulated over j tiles in PSUM)
  1/den                        one strided-partition DVE reciprocal_approx_fast per
                               pair straight off the av psum rows; bf16 recip rows
                               broadcast across partitions via a DRAM-hop DMA
  outT   = raw av^T copies (DVE, frees the av bank fast), then deferred bf16
           multiplies by the broadcast recip rows; project with w_out -> HBM
Batches are software-pipelined: batch b's x-load and qkv matmuls are issued at
staggered points inside batch b-1's attention rounds.
"""

import sys

if "/opt/trn_rl_repo" not in sys.path:
    sys.path.insert(0, "/opt/trn_rl_repo")

import numpy as np
import ml_dtypes

B, D, WS = 32, 256, 25
N = WS * WS            # 625
NP = 640               # padded row length (4B-aligned bf16 rows)
H, DH = 8, 32
NC = 8                 # cores
BL = B // NC           # 4 batches per core
SCALE = DH ** -0.5
JT = 5                 # j tiles of 125
JP = N // JT           # 125
CHUNKS = ((0, 512), (512, 113))   # i chunks, bank-aligned (matmul must not cross a 512-f32 PSUM bank)

_cache = {}


def _build():
    import concourse.bass as bass
    from concourse import bacc, mybir
    from concourse.tile import TileContext

    f32 = mybir.dt.float32
    bf16 = mybir.dt.bfloat16

    nc = bacc.Bacc()
    x_ext = nc.declare_dram_parameter("x", [BL, D, N], bf16, isOutput=False)
    wqkv_ext = nc.declare_dram_parameter("wqkv", [D, 3 * D], bf16, isOutput=False)
    wout_ext = nc.declare_dram_parameter("wout", [D, D], bf16, isOutput=False)
    biasT_ext = nc.declare_dram_parameter("biasT", [H, JT, JP, NP], bf16, isOutput=False)
    out_ext = nc.declare_dram_parameter("out", [BL, D, N], f32, isOutput=True)

    with TileContext(nc) as tc:
        with (
            tc.tile_pool(name="const", bufs=1) as const,
            tc.tile_pool(name="xp", bufs=2) as xp,
            tc.tile_pool(name="qk", bufs=2) as qkp,
            tc.tile_pool(name="vp", bufs=2) as vp,
            tc.tile_pool(name="es", bufs=4) as esp,
            tc.tile_pool(name="dn", bufs=3) as dnp,
            tc.tile_pool(name="avs", bufs=6) as avsp,
            tc.tile_pool(name="rb", bufs=6) as rbp,
            tc.tile_pool(name="ot", bufs=2) as otp,
            tc.tile_pool(name="res", bufs=2) as resp,
            tc.tile_pool(name="drp", bufs=6, space="DRAM") as drp,
            tc.tile_pool(name="sim", bufs=2, space="PSUM") as simp,
            tc.tile_pool(name="avb", bufs=1, space="PSUM") as avp,
            tc.tile_pool(name="acc", bufs=1, space="PSUM") as accp,
        ):
            wqkv_sb = const.tile([128, 2, 3 * D], bf16)
            nc.sync.dma_start(out=wqkv_sb, in_=wqkv_ext.rearrange("(k p) c -> p k c", p=128))
            wout_sb = const.tile([128, 2, D], bf16)
            nc.sync.dma_start(out=wout_sb, in_=wout_ext.rearrange("(k p) c -> p k c", p=128))
            biasT_sb = const.tile([JP, H, JT, NP], bf16)
            for h in range(H):
                for jt in range(JT):
                    nc.sync.dma_start(out=biasT_sb[:, h, jt, :], in_=biasT_ext[h, jt])

            def x_load(b):
                x_sb = xp.tile([128, 2, N], bf16, tag="x", name=f"x_sb_{b}")
                nc.sync.dma_start(out=x_sb, in_=x_ext[b].rearrange("(k p) n -> p k n", p=128))
                return x_sb

            def qkv_pieces(b, x_sb):
                """Closures, each issuing one slice of batch b's qkv work."""
                qkT_sb = qkp.tile([128, 4, NP], bf16, tag="qkT", name=f"qkT_{b}")
                v_sb = vp.tile([JP, JT, H, 2 * DH], bf16, tag="v", name=f"v_sb_{b}")

                def do_m(m):
                    ps = accp.tile([128, NP], f32, tag="acc", name=f"ps_{b}_{m}")
                    for kt in range(2):
                        for lo, sz in CHUNKS:
                            nc.tensor.matmul(
                                ps[:, lo:lo + sz],
                                wqkv_sb[:, kt, m * 128:(m + 1) * 128],
                                x_sb[:, kt, lo:lo + sz],
                                start=(kt == 0), stop=(kt == 1))
                    nc.vector.tensor_copy(qkT_sb[:, m, :N], ps[:, :N])

                def do_v(nt):
                    psv = accp.tile([128, NP], f32, tag="acc", name=f"psv_{b}_{nt}")
                    for kt in range(2):
                        nc.tensor.matmul(
                            psv[0:JP, :D],
                            x_sb[:, kt, nt * JP:(nt + 1) * JP],
                            wqkv_sb[:, kt, 2 * D:3 * D],
                            start=(kt == 0), stop=(kt == 1))
                    hv = psv[0:JP, :D].rearrange("p (h d) -> p h d", h=H)
                    # every head: ones col 0 (den -> av row 64*s), zeros cols
                    # 1:32, v at cols 32:64 (av rows 32:64 / 96:128) -- one
                    # full-tile reciprocal then covers both heads' den rows
                    nc.vector.tensor_copy(v_sb[:, nt, :, DH:2 * DH], hv)

                def do_consts():
                    nc.gpsimd.memset(v_sb[:, :, :, 0:1], 1.0)
                    nc.gpsimd.memset(v_sb[:, :, :, 1:DH], 0.0)

                pieces = [lambda m=m: do_m(m) for m in range(4)]
                pieces += [lambda nt=nt: do_v(nt) for nt in range(JT)]
                pieces.append(do_consts)
                return (b, qkT_sb, v_sb), pieces

            def attention_phase(ctx, hooks=None):
                """Attention + projection for a prepared batch. hooks[r] (if set)
                is invoked after round r to interleave the next batch's work."""
                b, qkT_sb, v_sb = ctx
                hooks = hooks or {}
                outT_sb = otp.tile([128, 2, NP], bf16, tag="outT", name=f"outT_{b}")

                av_box = [None]
                muls_pend = []

                def finish_pair(p, av):
                    # raw av -> SBUF bf16 (frees the av psum bank quickly)
                    av_sb = avsp.tile([128, 2, 512], bf16, tag="avsb", name=f"avsb_{b}_{p}")
                    nc.vector.tensor_copy(av_sb[:, 0, :], av[:, 0, :])
                    nc.vector.tensor_copy(av_sb[:, 1, 0:113], av[:, 1, 0:113])
                    # one chunked reciprocal covers both den rows (0, 64 of
                    # av_sb; junk rows are unused); bf16 den is plenty accurate
                    rcp_f = dnp.tile([128, 2, 512], f32, tag="rcpf", name=f"rcpf_{b}_{p}")
                    nc.vector.reciprocal(rcp_f[:, 0, :], av_sb[:, 0, :])
                    nc.vector.reciprocal(rcp_f[:, 1, 0:113], av_sb[:, 1, 0:113])
                    rcp_b = dnp.tile([128, 2, 512], bf16, tag="rcpb", name=f"rcpb_{b}_{p}")
                    nc.vector.tensor_copy(rcp_b[:, 0, :], rcp_f[:, 0, :])
                    nc.vector.tensor_copy(rcp_b[:, 1, 0:113], rcp_f[:, 1, 0:113])
                    nc.vector.memset(rcp_b[:, 1, 113:512], 1.0)
                    rbs = []
                    for s in range(2):
                        # recip rows: 0 = even head den, 64 = odd head den
                        rcp_d = drp.tile([1024], bf16, tag="rcpd",
                                         name=f"rcpd_{b}_{p}_{s}")
                        nc.sync.dma_start(out=rcp_d,
                                          in_=rcp_b[64 * s:64 * s + 1, :, :])
                        ap = bass.AP(tensor=rcp_d.tensor, offset=rcp_d.offset,
                                     ap=[[0, 128], [1, 1024]])
                        rb_s = rbp.tile([128, 2, 512], bf16, tag="rb",
                                        name=f"rb_{b}_{p}_{s}")
                        nc.sync.dma_start(out=rb_s, in_=ap)
                        rbs.append(rb_s)
                    muls_pend.append((p, av_sb, rbs))

                def issue_muls():
                    p, av_sb, rbs = muls_pend.pop(0)
                    for s in range(2):
                        h = 2 * p + s
                        hq, mt = (h % 4) * 32, h // 4
                        ro = 32 + 64 * s  # even head rows 32:64, odd 96:128
                        for ci, (lo, sz) in enumerate(CHUNKS):
                            nc.vector.tensor_mul(
                                outT_sb[hq:hq + DH, mt, lo:lo + sz],
                                av_sb[ro:ro + DH, ci, 0:sz],
                                rbs[s][ro:ro + DH, ci, 0:sz])

                def issue_av(p, jt, es_pair):
                    if jt == 0:
                        av_box[0] = avp.tile([128, 2, 512], f32, tag="av",
                                             name=f"av_{b}_{p}")
                    av = av_box[0]
                    for ci, (lo, sz) in enumerate(CHUNKS):
                        for s in range(2):
                            nc.tensor.matmul(
                                av[64 * s:64 * s + 2 * DH, ci, 0:sz],
                                v_sb[0:JP, jt, 2 * p + s, :],
                                es_pair[s][0:JP, lo:lo + sz],
                                start=(jt == 0), stop=(jt == JT - 1),
                                tile_position=(0, 64 * s),
                                skip_group_check=True)
                    if jt == JT - 1:
                        finish_pair(p, av)

                rounds = [(p, jt) for p in range(4) for jt in range(JT)]
                pend = []
                for r, (p, jt) in enumerate(rounds):
                    if r % JT == 3 and muls_pend:
                        issue_muls()
                    sims = [simp.tile([JP, NP], f32, tag="sim", name=f"sim_{b}_{r}_{s}")
                            for s in range(2)]
                    for ci, (lo, sz) in enumerate(CHUNKS):
                        for s in range(2):
                            h = 2 * p + s
                            hq, mt = (h % 4) * 32, h // 4
                            nc.tensor.matmul(
                                sims[s][:, lo:lo + sz],
                                qkT_sb[hq:hq + 32, 2 + mt, jt * JP:(jt + 1) * JP],
                                qkT_sb[hq:hq + 32, mt, lo:lo + sz],
                                start=True, stop=True, tile_position=(hq, 0))
                    es_pair = []
                    for s in range(2):
                        h = 2 * p + s
                        esr = esp.tile([JP, NP], bf16, tag="esr", name=f"esr_{b}_{r}_{s}")
                        nc.scalar.activation(out=esr[:, :N], in_=sims[s][:, :N],
                                             func=mybir.ActivationFunctionType.Exp)
                        es = esp.tile([JP, NP], bf16, tag="es", name=f"es_{b}_{r}_{s}")
                        eng = nc.gpsimd if (r + s) % 5 == 4 else nc.vector
                        eng.tensor_mul(es[:, :N], esr[:, :N],
                                       biasT_sb[0:JP, h, jt, :N])
                        es_pair.append(es)
                    pend.append((p, jt, es_pair))
                    if len(pend) > 1:
                        pp, pjt, pes = pend.pop(0)
                        issue_av(pp, pjt, pes)
                    for fcb in hooks.get(r, ()):
                        fcb()
                while pend:
                    pp, pjt, pes = pend.pop(0)
                    issue_av(pp, pjt, pes)

                def do_proj():
                    # output projection: out^T[c, i] = sum_d wout[d, c] outT[d, i]
                    for ct in range(2):
                        psp = accp.tile([128, NP], f32, tag="acc", name=f"psp_{b}_{ct}")
                        for kt in range(2):
                            for lo, sz in CHUNKS:
                                nc.tensor.matmul(
                                    psp[:, lo:lo + sz],
                                    wout_sb[:, kt, ct * 128:(ct + 1) * 128],
                                    outT_sb[:, kt, lo:lo + sz],
                                    start=(kt == 0), stop=(kt == 1))
                        o_t = resp.tile([128, NP], f32, tag="ot", name=f"o_t_{b}_{ct}")
                        nc.scalar.copy(o_t[:, :N], psp[:, :N])
                        nc.sync.dma_start(out=out_ext[b, ct * 128:(ct + 1) * 128, :],
                                          in_=o_t[:, :N])

                # finisher closures, to be run during the next batch's rounds
                return [issue_muls, do_proj]

            # software pipeline across batches: the previous batch's
            # recip/normalize/projection finishers run in rounds 0-3, the next
            # batch's x-load and qkv pieces are spread over rounds 1 and 4-13
            x0 = x_load(0)
            ctx, pieces = qkv_pieces(0, x0)
            for piece in pieces:
                piece()
            fin = None
            for b in range(1, BL + 1):
                hooks = {}
                if fin is not None:
                    for i, fcb in enumerate(fin):
                        hooks.setdefault(i, []).append(fcb)
                if b < BL:
                    box = {}

                    def mk_xload(bb=b, box=box):
                        box["x"] = x_load(bb)

                    def mk_qkv(bb=b, box=box):
                        box["ctx"], box["pieces"] = qkv_pieces(bb, box["x"])
                        box["pieces"][0]()

                    hooks.setdefault(1, []).append(mk_xload)
                    hooks.setdefault(4, []).append(mk_qkv)
                    for i in range(1, 10):
                        def run_piece(i=i, box=box):
                            box["pieces"][i]()
                        hooks.setdefault(4 + i, []).append(run_piece)
                fin = attention_phase(ctx, hooks)
                if b < BL:
                    ctx = box["ctx"]
            for fcb in fin:
                fcb()

    nc.compile()
    return nc


def _get_nc():
    if "nc" not in _cache:
        _cache["nc"] = _build()
    return _cache["nc"]


def make_in_maps(x, w_qkv, rel_emb, w_out, rel_idx):
    bf = ml_dtypes.bfloat16
    wqkv_s = np.array(w_qkv, dtype=np.float32, copy=True)
    wqkv_s[:, :D] *= SCALE                      # fold q scaling into weights
    wqkv_b = wqkv_s.astype(bf)
    wout_b = np.asarray(w_out, dtype=np.float32).astype(bf)
    # bias[h, i, j] = rel_emb[rel_idx[i, j], h];  biasT[h, j, i] = bias[h, i, j]
    bias = np.asarray(rel_emb, dtype=np.float32)[np.asarray(rel_idx)]   # [i, j, h]
    ebiasT = np.exp(np.ascontiguousarray(bias.transpose(2, 1, 0)))      # [h, j, i]
    biasT = np.zeros((H, JT, JP, NP), dtype=np.float32)
    biasT[..., :N] = ebiasT.reshape(H, JT, JP, N)
    biasT = biasT.astype(bf)
    xf = np.asarray(x, dtype=np.float32).reshape(B, D, N).astype(bf)
    return [
        {"x": xf[c * BL:(c + 1) * BL], "wqkv": wqkv_b, "wout": wout_b, "biasT": biasT}
        for c in range(NC)
    ]


def kernel(x, w_qkv, rel_emb, w_out, rel_idx):
    from concourse.bass_utils import run_bass_kernel_spmd

    nc = _get_nc()
    in_maps = make_in_maps(x, w_qkv, rel_emb, w_out, rel_idx)
    res = run_bass_kernel_spmd(nc, in_maps, list(range(NC)))
    out = np.concatenate([res.results[c]["out"] for c in range(NC)], axis=0)
    return out.reshape(B, D, WS, WS).astype(np.float32)
